# revision 6
# baseline (speedup 1.0000x reference)
"""Trainium2 Bass kernel for nn_AdaptiveLSTM (2-layer LSTM with modality input).

Strategy: tensor-parallel over the 4H gate dimension across 8 NeuronCores.
Each core owns a 512-column gate shard (128 columns of each gate, local order
[f, i, o, g]) which corresponds to a 128-wide slice of the hidden dim. Per
step, one combined AllGather exchanges [h0T(t), h1T(t-1)] tiles ([128, 64]
each, transposed so h-dim is on partitions = matmul stationary layout).

Matmuls run in float32r (full-rate fp32 on the PE array). Input projections
x@W0 + z@V0 + bias and z@V1 + bias are precomputed time-parallel into DRAM
streams, consumed per step. The fc head partial product is accumulated on the
fly; partials are summed on the host.
"""
import os
import sys

os.environ.setdefault("BASS_DISABLE_FRAME_TO_TRACEBACK", "1")
sys.path.insert(0, "/opt/trn_rl_repo")

import numpy as np  # noqa: E402

B, S, D_IN, H, D_ADD, OUT = 64, 512, 512, 1024, 128, 3
NCORES = 8
# local gate block order [f, i, o, g] -> reference gate index (f, i, g, o)
GATE_OF_BLOCK = [0, 1, 3, 2]


def build_nc(n_steps):
    from concourse import bass, bacc, mybir, tile

    FP = mybir.dt.float32
    FPR = mybir.dt.float32r
    n_tiles = n_steps // 2
    assert n_steps % 2 == 0
    YCHUNK = min(32, n_steps)

    nc = bacc.Bacc(None)
    # --- parameters (all fp32-typed numpy on host; FPR marks matmul operands)
    xt_p = nc.declare_dram_parameter("xt", [n_tiles, 128, 4, 128], FPR, False)
    zt_p = nc.declare_dram_parameter("zt", [n_tiles, 128, 128], FPR, False)
    w0_p = nc.declare_dram_parameter("w0s", [128, 4, 512], FPR, False)
    v0_p = nc.declare_dram_parameter("v0s", [128, 512], FPR, False)
    v1_p = nc.declare_dram_parameter("v1s", [128, 512], FPR, False)
    u0_p = nc.declare_dram_parameter("u0s", [128, 8, 512], FPR, False)
    w1_p = nc.declare_dram_parameter("w1s", [128, 8, 512], FPR, False)
    u1_p = nc.declare_dram_parameter("u1s", [128, 8, 512], FPR, False)
    b0_p = nc.declare_dram_parameter("b0r", [128, 512], FP, False)
    b1_p = nc.declare_dram_parameter("b1r", [128, 512], FP, False)
    fc_p = nc.declare_dram_parameter("fcw", [128, OUT], FPR, False)
    id_p = nc.declare_dram_parameter("idty", [64, 64], FP, False)
    y_o = nc.declare_dram_parameter("yT", [OUT, 64 * n_steps], FP, True)
    hc_o = nc.declare_dram_parameter("hc", [4, 64, 128], FP, True)

    p0d = nc.dram_tensor("p0d", [n_tiles, 128, 512], FP)
    q1d = nc.dram_tensor("q1d", [n_tiles, 128, 512], FP)

    Sig = mybir.ActivationFunctionType.Sigmoid
    Tanh = mybir.ActivationFunctionType.Tanh

    with tile.TileContext(nc) as tc:
        with tc.tile_pool(name="consts", bufs=1) as cst:
            w0sb = cst.tile([128, 4, 512], FPR, name="w0sb")
            v0sb = cst.tile([128, 512], FPR, name="v0sb")
            v1sb = cst.tile([128, 512], FPR, name="v1sb")
            u0sb = cst.tile([128, 8, 512], FPR, name="u0sb")
            w1sb = cst.tile([128, 8, 512], FPR, name="w1sb")
            u1sb = cst.tile([128, 8, 512], FPR, name="u1sb")
            b0sb = cst.tile([128, 512], FP, name="b0sb")
            b1sb = cst.tile([128, 512], FP, name="b1sb")
            fcsb = cst.tile([128, OUT], FPR, name="fcsb")
            idsb = cst.tile([64, 64], FP, name="idsb")
            for dst, src in [(w0sb, w0_p), (v0sb, v0_p), (v1sb, v1_p),
                             (u0sb, u0_p), (w1sb, w1_p), (u1sb, u1_p),
                             (b0sb, b0_p), (b1sb, b1_p), (fcsb, fc_p),
                             (idsb, id_p)]:
                nc.sync.dma_start(out=dst[...], in_=src[...])

            # ---------- phase 1: time-parallel input projections ----------
            with tc.tile_pool(name="pc", bufs=3) as pc, \
                 tc.tile_pool(name="pcps", bufs=2, space="PSUM") as pcps:
                for ti in range(n_tiles):
                    xtile = pc.tile([128, 4, 128], FPR, tag="xtile", name="xtile")
                    ztile = pc.tile([128, 128], FPR, tag="ztile", name="ztile")
                    nc.sync.dma_start(out=xtile[...], in_=xt_p[ti])
                    nc.sync.dma_start(out=ztile[...], in_=zt_p[ti])
                    ps0 = pcps.tile([128, 512], FP, tag="ps0", name="ps0")
                    for k in range(4):
                        nc.tensor.matmul(ps0[...], lhsT=xtile[:, k, :],
                                         rhs=w0sb[:, k, :], start=(k == 0),
                                         stop=False)
                    nc.tensor.matmul(ps0[...], lhsT=ztile[...], rhs=v0sb[...],
                                     start=False, stop=True)
                    ps1 = pcps.tile([128, 512], FP, tag="ps1", name="ps1")
                    nc.tensor.matmul(ps1[...], lhsT=ztile[...], rhs=v1sb[...],
                                     start=True, stop=True)
                    p0t = pc.tile([128, 512], FP, tag="p0t", name="p0t")
                    nc.vector.tensor_add(out=p0t[...], in0=ps0[...], in1=b0sb[...])
                    q1t = pc.tile([128, 512], FP, tag="q1t", name="q1t")
                    nc.vector.tensor_add(out=q1t[...], in0=ps1[...], in1=b1sb[...])
                    nc.sync.dma_start(out=p0d[ti], in_=p0t[...])
                    nc.sync.dma_start(out=q1d[ti], in_=q1t[...])

            # ---------- phase 2: recurrence ----------
            with tc.tile_pool(name="st", bufs=1) as st, \
                 tc.tile_pool(name="rec", bufs=3) as rec, \
                 tc.tile_pool(name="hh", bufs=2) as hh, \
                 tc.tile_pool(name="rps", bufs=1, space="PSUM") as rps, \
                 tc.tile_pool(name="dram", bufs=2, space="DRAM") as drm:
                c0 = st.tile([64, 128], FP, name="c0")
                c1 = st.tile([64, 128], FP, name="c1")
                hself = st.tile([128, 2, 128], FPR, name="hself")
                nc.vector.memset(c0[...], 0.0)
                nc.vector.memset(c1[...], 0.0)
                nc.vector.memset(hself[:, 0, 64:128].bitcast(FP), 0.0)

                h0all_prev = None
                h0_t = c0  # placeholders for final-output refs
                h1_t = c1

                for t in range(n_steps):
                    ti, half = t // 2, (t % 2) * 64
                    # layer-0 pre-activation
                    p0t = rec.tile([64, 512], FP, tag="p0t2", name="p0t2")
                    nc.sync.dma_start(out=p0t[...],
                                      in_=p0d[ti, half:half + 64, :])
                    if t > 0:
                        psum0 = rps.tile([64, 512], FP, tag="pre0", name="pre0", bufs=2)
                        for k in range(8):
                            nc.tensor.matmul(psum0[...],
                                             lhsT=h0all_prev[:, k, :],
                                             rhs=u0sb[:, k, :],
                                             start=(k == 0), stop=(k == 7))
                        pre0 = rec.tile([64, 512], FP, tag="pre0sb", name="pre0sb")
                        nc.vector.tensor_add(out=pre0[...], in0=psum0[...],
                                             in1=p0t[...])
                    else:
                        pre0 = p0t
                    # gates layer 0: local col order [f, i, o, g]
                    gs0 = rec.tile([64, 512], FP, tag="gs0", name="gs0")
                    nc.scalar.activation(gs0[:, 0:384], pre0[:, 0:384], Sig)
                    nc.scalar.activation(gs0[:, 384:512], pre0[:, 384:512], Tanh)
                    ig0 = rec.tile([64, 128], FP, tag="ig0", name="ig0")
                    nc.vector.tensor_mul(out=ig0[...], in0=gs0[:, 128:256],
                                         in1=gs0[:, 384:512])
                    nc.vector.tensor_mul(out=c0[...], in0=gs0[:, 0:128],
                                         in1=c0[...])
                    nc.vector.tensor_add(out=c0[...], in0=c0[...], in1=ig0[...])
                    tc0 = rec.tile([64, 128], FP, tag="tc0", name="tc0")
                    nc.scalar.activation(tc0[...], c0[...], Tanh)
                    h0 = rec.tile([64, 128], FP, tag="h0", name="h0")
                    nc.vector.tensor_mul(out=h0[...], in0=gs0[:, 256:384],
                                         in1=tc0[...])
                    h0_t = h0
                    # transpose h0 -> hself[:, t%2, 0:64]
                    tp0 = rps.tile([128, 64], FP, tag="tp0", name="tp0", bufs=1)
                    nc.tensor.transpose(tp0[...], h0[...], idsb[...])
                    nc.vector.tensor_copy(out=hself[:, t % 2, 0:64], in_=tp0[...])
                    # combined AllGather of [h0T(t), h1T(t-1)]
                    agin = drm.tile([128, 128], FPR, tag="agin", name="agin")
                    agout = drm.tile([NCORES * 128, 128], FPR, tag="agout",
                                     name="agout")
                    nc.sync.dma_start(out=agin[...], in_=hself[:, t % 2, :])
                    nc.gpsimd.collective_compute(
                        "AllGather", mybir.AluOpType.bypass,
                        ins=[agin[...].opt()], outs=[agout[...].opt()],
                        replica_groups=[list(range(NCORES))],
                    )
                    h0all = hh.tile([128, 8, 64], FPR, tag="h0all", name="h0all")
                    h1all = hh.tile([128, 8, 64], FPR, tag="h1all", name="h1all")
                    agv = agout[...].rearrange("(r p) f -> p r f", r=NCORES)
                    nc.sync.dma_start(out=h0all[...], in_=agv[:, :, 0:64])
                    nc.sync.dma_start(out=h1all[...], in_=agv[:, :, 64:128])
                    # layer-1 pre-activation
                    q1t = rec.tile([64, 512], FP, tag="q1t2", name="q1t2")
                    nc.sync.dma_start(out=q1t[...],
                                      in_=q1d[ti, half:half + 64, :])
                    psum1 = rps.tile([64, 512], FP, tag="pre1", name="pre1", bufs=2)
                    for k in range(8):
                        nc.tensor.matmul(psum1[...], lhsT=h0all[:, k, :],
                                         rhs=w1sb[:, k, :], start=(k == 0),
                                         stop=(t == 0 and k == 7))
                    if t > 0:
                        for k in range(8):
                            nc.tensor.matmul(psum1[...], lhsT=h1all[:, k, :],
                                             rhs=u1sb[:, k, :], start=False,
                                             stop=(k == 7))
                    pre1 = rec.tile([64, 512], FP, tag="pre1sb", name="pre1sb")
                    nc.vector.tensor_add(out=pre1[...], in0=psum1[...],
                                         in1=q1t[...])
                    gs1 = rec.tile([64, 512], FP, tag="gs1", name="gs1")
                    nc.scalar.activation(gs1[:, 0:384], pre1[:, 0:384], Sig)
                    nc.scalar.activation(gs1[:, 384:512], pre1[:, 384:512], Tanh)
                    ig1 = rec.tile([64, 128], FP, tag="ig1", name="ig1")
                    nc.vector.tensor_mul(out=ig1[...], in0=gs1[:, 128:256],
                                         in1=gs1[:, 384:512])
                    nc.vector.tensor_mul(out=c1[...], in0=gs1[:, 0:128],
                                         in1=c1[...])
                    nc.vector.tensor_add(out=c1[...], in0=c1[...], in1=ig1[...])
                    tc1 = rec.tile([64, 128], FP, tag="tc1", name="tc1")
                    nc.scalar.activation(tc1[...], c1[...], Tanh)
                    h1 = rec.tile([64, 128], FP, tag="h1", name="h1")
                    nc.vector.tensor_mul(out=h1[...], in0=gs1[:, 256:384],
                                         in1=tc1[...])
                    h1_t = h1
                    # transpose h1 -> hself[:, (t+1)%2, 64:128] for next AG
                    tp1 = rps.tile([128, 64], FP, tag="tp1", name="tp1", bufs=1)
                    nc.tensor.transpose(tp1[...], h1[...], idsb[...])
                    nc.vector.tensor_copy(out=hself[:, (t + 1) % 2, 64:128],
                                          in_=tp1[...])
                    # fc head partial: yT[:, t*64:(t+1)*64] += fcw.T @ h1T
                    psy = rps.tile([OUT, 64], FP, tag="psy", name="psy", bufs=1)
                    nc.tensor.matmul(psy[...], lhsT=fcsb[...],
                                     rhs=hself[:, (t + 1) % 2, 64:128],
                                     start=True, stop=True)
                    if t % YCHUNK == 0:
                        ysb = rec.tile([OUT, 64 * YCHUNK], FP, tag="ysb",
                                       name="ysb", bufs=2)
                    nc.vector.tensor_copy(
                        out=ysb[:, (t % YCHUNK) * 64:(t % YCHUNK + 1) * 64],
                        in_=psy[...])
                    if t % YCHUNK == YCHUNK - 1 or t == n_steps - 1:
                        base = (t // YCHUNK) * 64 * YCHUNK
                        nc.sync.dma_start(
                            out=y_o[:, base:base + 64 * (t % YCHUNK + 1)],
                            in_=ysb[:, :64 * (t % YCHUNK + 1)])
                    h0all_prev = h0all

                nc.sync.dma_start(out=hc_o[0], in_=h0_t[...])
                nc.sync.dma_start(out=hc_o[1], in_=c0[...])
                nc.sync.dma_start(out=hc_o[2], in_=h1_t[...])
                nc.sync.dma_start(out=hc_o[3], in_=c1[...])
    nc.finalize()
    return nc


def shard_cols(core):
    """Global 4H column indices for core's local [f, i, o, g] shard."""
    cols = np.zeros(512, dtype=np.int64)
    for blk, gate in enumerate(GATE_OF_BLOCK):
        cols[blk * 128:(blk + 1) * 128] = gate * H + core * 128 + np.arange(128)
    return cols


def prep_inputs(x, z, W0, bW0, U0, V0, b0, W1, bW1, U1, V1, b1, fc_W,
                n_steps):
    n_tiles = n_steps // 2
    # token order s*64 + b
    xs = np.ascontiguousarray(x.transpose(1, 0, 2)).reshape(S * B, D_IN)
    zs = np.ascontiguousarray(z.transpose(1, 0, 2)).reshape(S * B, D_ADD)
    xt = np.ascontiguousarray(
        xs[:n_tiles * 128].reshape(n_tiles, 128, 4, 128).transpose(0, 3, 2, 1))
    zt = np.ascontiguousarray(
        zs[:n_tiles * 128].reshape(n_tiles, 128, 128).transpose(0, 2, 1))
    idty = np.eye(64, dtype=np.float32)
    bias0 = bW0 + b0
    bias1 = bW1 + b1
    in_maps = []
    for c in range(NCORES):
        cols = shard_cols(c)
        w0c = np.ascontiguousarray(
            W0[:, cols].reshape(4, 128, 512).transpose(1, 0, 2))
        u0c = np.ascontiguousarray(
            U0[:, cols].reshape(8, 128, 512).transpose(1, 0, 2))
        w1c = np.ascontiguousarray(
            W1[:, cols].reshape(8, 128, 512).transpose(1, 0, 2))
        u1c = np.ascontiguousarray(
            U1[:, cols].reshape(8, 128, 512).transpose(1, 0, 2))
        in_maps.append({
            "xt": xt, "zt": zt,
            "w0s": w0c,
            "v0s": np.ascontiguousarray(V0[:, cols]),
            "v1s": np.ascontiguousarray(V1[:, cols]),
            "u0s": u0c, "w1s": w1c, "u1s": u1c,
            "b0r": np.tile(bias0[cols], (128, 1)).astype(np.float32),
            "b1r": np.tile(bias1[cols], (128, 1)).astype(np.float32),
            "fcw": np.ascontiguousarray(fc_W[c * 128:(c + 1) * 128, :]),
            "idty": idty,
        })
    return in_maps


def assemble_outputs(results, fc_b, n_steps):
    yT = np.zeros((OUT, 64 * n_steps), dtype=np.float64)
    for r in results:
        yT += r["yT"].astype(np.float64)
    # token order s*64+b -> y[b, s, o]
    y = (yT.reshape(OUT, n_steps, B).transpose(2, 1, 0) +
         fc_b.astype(np.float64)).astype(np.float32)
    h0 = np.zeros((B, H), dtype=np.float32)
    c0 = np.zeros((B, H), dtype=np.float32)
    h1 = np.zeros((B, H), dtype=np.float32)
    c1 = np.zeros((B, H), dtype=np.float32)
    for c, r in enumerate(results):
        hc = r["hc"]
        h0[:, c * 128:(c + 1) * 128] = hc[0]
        c0[:, c * 128:(c + 1) * 128] = hc[1]
        h1[:, c * 128:(c + 1) * 128] = hc[2]
        c1[:, c * 128:(c + 1) * 128] = hc[3]
    hN = np.stack([h0, h1])
    cN = np.stack([c0, c1])
    return y, (hN, cN)


def run(inputs, n_steps=S, trace=False):
    """Build + run on hardware; returns (results_list, exec_time_ns)."""
    from concourse.bass_utils import run_bass_kernel_spmd
    in_maps = prep_inputs(
        inputs["x"], inputs["z"], inputs["W0"], inputs["bW0"], inputs["U0"],
        inputs["V0"], inputs["b0"], inputs["W1"], inputs["bW1"], inputs["U1"],
        inputs["V1"], inputs["b1"], inputs["fc_W"], n_steps)
    nc = build_nc(n_steps)
    res = run_bass_kernel_spmd(nc, in_maps, list(range(NCORES)), trace=trace)
    return res


def kernel(**inputs):
    inputs = {k: np.asarray(v, dtype=np.float32) for k, v in inputs.items()}
    res = run(inputs, n_steps=S, trace=False)
    return assemble_outputs(res.results, inputs["fc_b"], S)


if __name__ == "__main__":
    pass
